# revision 14
# baseline (speedup 1.0000x reference)
"""Trainium2 Bass kernel for a dense transformer block.

Reference math (B=32, S=577, D=768, H=12, DH=64, F=3072, fp32):
  h  = LN1(x);  q,k,v = per-head projections of h
  scores = q @ k^T / sqrt(DH)
  probs  = softmax(scores, axis=QUERY)       # quirk: softmax over the query axis
  attn   = probs @ v;  x2 = x + concat(attn) @ Wo + bo
  out    = x2 + (gelu(LN2(x2) @ W1 + b1) @ W2 + b2)

Strategy: pure data-parallel over batch, 4 batch items per core on 8 cores, no
collectives.  On-chip activations live transposed [feature on partitions, token
on free dim].  v2: fp16 residual stream end-to-end (x shipped fp16, halved DMA
and 2x DVE rate), fp8e4m3 DoubleRow matmuls (K=256/instr, 2x PE throughput) for
the QKV projections, output projection and FC2 (FC1 optionally), with
power-of-2 scales (weights x128, normalized activations x16) descaled in the
existing PSUM->SBUF copy ops.  Dual-fp8 ldweights requires the two k-subtiles
contiguous in SBUF, so fp8 weights ship pre-packed as [128, pair, block, 2,
128] and V is projected transposed (like Q/K) then PE-transposed back to the
natural [token, feature] layout the attention kernel needs.  LN stats run as
fp16 ones-matmuls.  bo is folded into a second host-side residual copy of x;
b2 is applied on the host after the gather.
"""

import numpy as np
import ml_dtypes

B, S, D, H, DH, F = 32, 577, 768, 12, 64, 3072
NCORES = 8
BPC = B // NCORES          # batches per core
EPS = 1e-5
NCD = D // 128             # 6  d-chunks
NCF = F // 128             # 24 f-chunks
NHP = H // 2               # 6  head pairs
PD = D // 256              # 3  d chunk-pairs
PF = F // 256              # 12 f chunk-pairs
SSPL = [(0, 512), (512, S - 512)]              # free-dim splits of S for matmul/psum
TCH = [(i * 128, min(128, S - i * 128)) for i in range((S + 127) // 128)]  # 5 t-chunks
S1 = S + (S % 2)           # even-padded free dim for fp8 DoubleRow operand tiles

SW = 128.0                 # fp8 weight scale (power of 2)
SZ = 16.0                  # fp8 normalized-activation scale
DSC_QKV = 1.0 / (SW * SZ)  # psum descale for z1(fp8) @ w(fp8)
DSC_FC2 = 1.0 / SW         # psum descale for g(fp8, unscaled) @ w2(fp8)

_NC_CACHE = {}


def _build_nc(gelu_kind: str = "gelu", bpc: int = BPC, fc1_fp8: bool = False):
    from contextlib import ExitStack
    import concourse.bass as bass
    import concourse.tile as tile
    from concourse import bacc, mybir

    f32, fp16 = mybir.dt.float32, mybir.dt.float16
    fp8 = mybir.dt.float8e4
    AF = mybir.ActivationFunctionType
    ALU = mybir.AluOpType
    DR = mybir.MatmulPerfMode.DoubleRow
    GELU = {"gelu": AF.Gelu, "tanh": AF.Tanh}[gelu_kind]
    z2scale = SZ if fc1_fp8 else 1.0
    dsc_fc1 = DSC_QKV if fc1_fp8 else 1.0

    nc = bacc.Bacc("TRN2", target_bir_lowering=False, dynamic_dma_scratch_size=2048)
    xT_d = nc.declare_dram_parameter("xT", [bpc, D, S], fp16, isOutput=False)
    xTb_d = nc.declare_dram_parameter("xTb", [bpc, D, S], fp16, isOutput=False)
    wq_d = nc.declare_dram_parameter("wq", [128, PD, NCD, 2, 128], fp8, isOutput=False)
    wk_d = nc.declare_dram_parameter("wk", [128, PD, NCD, 2, 128], fp8, isOutput=False)
    wv_d = nc.declare_dram_parameter("wv", [128, PD, NCD, 2, 128], fp8, isOutput=False)
    wo_d = nc.declare_dram_parameter("wo", [128, PD, NCD, 2, 128], fp8, isOutput=False)
    if fc1_fp8:
        w1_d = nc.declare_dram_parameter("w1", [128, PD, NCF, 2, 128], fp8, isOutput=False)
    else:
        w1_d = nc.declare_dram_parameter("w1", [D, F], fp16, isOutput=False)
    w2_d = nc.declare_dram_parameter("w2", [128, PF, NCD, 2, 128], fp8, isOutput=False)
    bq_d = nc.declare_dram_parameter("bq", [NCD, 128], f32, isOutput=False)
    bk_d = nc.declare_dram_parameter("bk", [NCD, 128], f32, isOutput=False)
    bv_d = nc.declare_dram_parameter("bv", [NCD, 128], f32, isOutput=False)
    b1_d = nc.declare_dram_parameter("b1", [NCF, 128], f32, isOutput=False)
    idn_d = nc.declare_dram_parameter("idn", [128, 128], fp16, isOutput=False)
    outT_d = nc.declare_dram_parameter("outT", [bpc, D, S], fp16, isOutput=True)

    with tile.TileContext(nc) as tc:
        with ExitStack() as ctx:
            wp = ctx.enter_context(tc.tile_pool(name="wp", bufs=1))
            rp = ctx.enter_context(tc.tile_pool(name="rp", bufs=3))      # residual fp16
            zp = ctx.enter_context(tc.tile_pool(name="zp", bufs=1))      # normalized
            qkp = ctx.enter_context(tc.tile_pool(name="qkp", bufs=1))    # qt/kt/vt/v/concat
            ep = ctx.enter_context(tc.tile_pool(name="ep", bufs=2))      # exp tiles
            gp = ctx.enter_context(tc.tile_pool(name="gp", bufs=1))      # gelu acts
            sp_ = ctx.enter_context(tc.tile_pool(name="sp", bufs=1))     # small stat rows
            tp = ctx.enter_context(tc.tile_pool(name="tp", bufs=1))      # [128,S] temps
            mmp = ctx.enter_context(tc.tile_pool(name="mmp", bufs=4, space="PSUM"))

            # ---- weights / constants (resident); DMAs deferred until after
            # the first x-shard load so compute starts immediately ----
            wq_s = wp.tile([128, PD, NCD, 2, 128], fp8, name="wq_s")
            wk_s = wp.tile([128, PD, NCD, 2, 128], fp8, name="wk_s")
            wv_s = wp.tile([128, PD, NCD, 2, 128], fp8, name="wv_s")
            wo_s = wp.tile([128, PD, NCD, 2, 128], fp8, name="wo_s")
            if fc1_fp8:
                w1_s = wp.tile([128, PD, NCF, 2, 128], fp8, name="w1_s")
            else:
                w1_s = wp.tile([128, NCD, F], fp16, name="w1_s")
            w2_s = wp.tile([128, PF, NCD, 2, 128], fp8, name="w2_s")

            def emit_load_weights():
                nc.sync.dma_start(out=wq_s[:, :, :, :, :], in_=wq_d[:, :, :, :, :])
                nc.sync.dma_start(out=wk_s[:, :, :, :, :], in_=wk_d[:, :, :, :, :])
                nc.sync.dma_start(out=wv_s[:, :, :, :, :], in_=wv_d[:, :, :, :, :])
                nc.sync.dma_start(out=wo_s[:, :, :, :, :], in_=wo_d[:, :, :, :, :])
                if fc1_fp8:
                    nc.sync.dma_start(out=w1_s[:, :, :, :, :], in_=w1_d[:, :, :, :, :])
                else:
                    nc.sync.dma_start(out=w1_s[:, :, :], in_=w1_d.ap().rearrange("(c p) n -> p c n", p=128))
                nc.sync.dma_start(out=w2_s[:, :, :, :, :], in_=w2_d[:, :, :, :, :])
            bqs = wp.tile([128, NCD], f32, name="bqs")
            nc.sync.dma_start(out=bqs[:, :], in_=bq_d.ap().rearrange("c p -> p c"))
            bks = wp.tile([128, NCD], f32, name="bks")
            nc.sync.dma_start(out=bks[:, :], in_=bk_d.ap().rearrange("c p -> p c"))
            bvs = wp.tile([128, NCD], f32, name="bvs")
            nc.sync.dma_start(out=bvs[:, :], in_=bv_d.ap().rearrange("c p -> p c"))
            b1s = wp.tile([128, NCF], f32, name="b1s")
            nc.sync.dma_start(out=b1s[:, :], in_=b1_d.ap().rearrange("c p -> p c"))
            idn_s = wp.tile([128, 128], fp16, name="idn_s")
            nc.sync.dma_start(out=idn_s[:, :], in_=idn_d[:, :])
            ones128 = wp.tile([128, 1], fp16, name="ones128")
            nc.vector.memset(ones128[:, :], 1.0)
            ones1 = wp.tile([1, 128], fp16, name="ones1")
            nc.vector.memset(ones1[:, :], 1.0)
            sixteen1 = wp.tile([1, 128], fp16, name="sixteen1")
            nc.vector.memset(sixteen1[:, :], SZ)
            eps_s = wp.tile([1, 1], f32, name="eps_s")
            nc.vector.memset(eps_s[:, :], EPS)

            # ---------------- helpers ----------------
            def emit_squares(src, c):
                """ACT square of one chunk of src (fp16) -> fp16 tile for sumsq."""
                sq = tp.tile([128, S], fp16, name="sq", tag="castsq", bufs=2)
                nc.scalar.activation(sq[:, :], src[:, c, :], AF.Square)
                return sq

            def emit_stats(src):
                """Column sums & sums of squares of src [128, NCD, S] fp16 over
                the partition (feature) axis -> psum rows [0]=sum, [32]=sumsq."""
                spt = mmp.tile([128, S], f32, name="spt", tag="mm", padded_shape=[128, 1024])
                sqs = [emit_squares(src, c) for c in range(NCD)]
                for c in range(NCD):
                    for (s0, sn) in SSPL:
                        nc.tensor.matmul(spt[0:1, s0:s0 + sn], ones128[:, :],
                                         src[:, c, s0:s0 + sn],
                                         start=(c == 0), stop=(c == NCD - 1))
                        nc.tensor.matmul(spt[32:33, s0:s0 + sn], ones128[:, :],
                                         sqs[c][:, s0:s0 + sn],
                                         start=(c == 0), stop=(c == NCD - 1))
                return spt

            def emit_chain(spt, sfx):
                """LN scalar chain on [1,S] rows, minimized for serial depth:
                mu2 -> var -> rstd(ACT Rsqrt) -> nmr, with mu in parallel.
                The fp8 activation scale is folded into the bcast lhsT."""
                mu_s = sp_.tile([1, S], f32, name="mu_s", tag="mu_s" + sfx)
                nc.vector.tensor_scalar_mul(mu_s[:, :], spt[0:1, :], 1.0 / D)
                mu2 = sp_.tile([1, S], f32, name="mu2", tag="v_s" + sfx)
                nc.vector.tensor_mul(mu2[:, :], mu_s[:, :], mu_s[:, :])
                var = tp.tile([1, S], f32, name="var", tag="castsq", bufs=2)
                nc.vector.scalar_tensor_tensor(var[:, :], spt[32:33, :], 1.0 / D,
                                               mu2[:, :], op0=ALU.mult, op1=ALU.subtract)
                w_s = tp.tile([1, S], f32, name="w_s", tag="castsq", bufs=2)
                nc.scalar.activation(w_s[:, :], var[:, :], AF.Sqrt, bias=eps_s[0:1, 0:1])
                v_s = sp_.tile([1, S], f32, name="v_s2", tag="v_s2" + sfx)
                nc.vector.reciprocal_approx_fast(v_s[:, :], w_s[:, :])
                rstd_bf = sp_.tile([1, S], fp16, name="rstd_bf", tag="rstdbf" + sfx)
                nc.scalar.activation(rstd_bf[:, :], v_s[:, :], AF.Copy)
                nmr_bf = sp_.tile([1, S], fp16, name="nmr_bf", tag="nmrbf" + sfx)
                nc.vector.scalar_tensor_tensor(nmr_bf[:, :], mu_s[:, :], -1.0, v_s[:, :],
                                               op0=ALU.mult, op1=ALU.mult)
                return rstd_bf, nmr_bf

            def emit_bcast(row_bf, lhs_row):
                """Broadcast a [1,S] fp16 row across 128 partitions via rank-1
                matmul; lhs_row's value carries the fp8 activation scale."""
                bc = mmp.tile([128, D], f32, name="bc", tag="mm")
                for (s0, sn) in SSPL:
                    nc.tensor.matmul(bc[:, s0:s0 + sn], lhs_row[0:1, :], row_bf[0:1, s0:s0 + sn],
                                     start=True, stop=True)
                return bc

            def emit_znorm(src, rstd_bc, nmr_bc, z):
                """z[:,c,:] = (src[:,c,:] * rstd) + negmurstd."""
                for c in range(NCD):
                    zt = tp.tile([128, S], fp16, name="zt", tag="castsq", bufs=2)
                    nc.vector.tensor_mul(zt[:, :], src[:, c, :], rstd_bc[:, 0:S])
                    nc.vector.tensor_add(z[:, c, 0:S], zt[:, :], nmr_bc[:, 0:S])

            # ---------------- phase emitters ----------------
            xt_tiles = [None] * bpc
            z1_tiles = [None] * bpc
            z2_tiles = [None] * bpc
            x2_tiles = [None] * bpc
            ch1 = [None] * bpc
            ch2 = [None] * bpc

            def emit_load_x(b):
                xt = rp.tile([128, NCD, S], fp16, name="xt", tag="res")
                for c in range(NCD):
                    nc.sync.dma_start(out=xt[:, c, :],
                                      in_=xT_d[b, c * 128:(c + 1) * 128, :])
                xt_tiles[b] = xt

            def emit_stats1(b):
                ch1[b] = emit_chain(emit_stats(xt_tiles[b]), "1")

            def emit_zfinish1(b):
                rstd_bf, nmr_bf = ch1[b]
                rbc = emit_bcast(rstd_bf, sixteen1)
                nbc = emit_bcast(nmr_bf, sixteen1)
                z1 = zp.tile([128, NCD, S1], fp8, name="z1", tag="z")
                emit_znorm(xt_tiles[b], rbc, nbc, z1)
                z1_tiles[b] = z1

            def emit_attention(b, interleave=()):
                """interleave: callables emitted between head-pairs to feed the
                PE while ACT grinds the softmax exps."""
                interleave = list(interleave)
                z1 = z1_tiles[b]
                # --- QKV projections (fp8 DoubleRow over chunk pairs) ---
                qt = qkp.tile([128, NHP, S], fp16, name="qt", tag="qt")
                kt = qkp.tile([128, NHP, S], fp16, name="kt", tag="kt")
                vt = qkp.tile([128, NHP, S], fp16, name="vt", tag="vt")
                for hp in range(NHP):
                    qps = mmp.tile([128, S], f32, name="qps", tag="mm")
                    for (s0, sn) in SSPL:
                        for p in range(PD):
                            nc.tensor.matmul(qps[:, s0:s0 + sn], wq_s[:, p, hp, :, :],
                                             z1[:, 2 * p:2 * p + 2, s0:s0 + sn],
                                             start=(p == 0), stop=(p == PD - 1),
                                             perf_mode=DR)
                    nc.vector.tensor_scalar(qt[:, hp, :], qps[:, 0:S], DSC_QKV,
                                            bqs[:, hp:hp + 1], op0=ALU.mult, op1=ALU.add)
                    kps = mmp.tile([128, S], f32, name="kps", tag="mm")
                    for (s0, sn) in SSPL:
                        for p in range(PD):
                            nc.tensor.matmul(kps[:, s0:s0 + sn], wk_s[:, p, hp, :, :],
                                             z1[:, 2 * p:2 * p + 2, s0:s0 + sn],
                                             start=(p == 0), stop=(p == PD - 1),
                                             perf_mode=DR)
                    nc.vector.tensor_scalar(kt[:, hp, :], kps[:, 0:S], DSC_QKV,
                                            bks[:, hp:hp + 1], op0=ALU.mult, op1=ALU.add)
                    vps = mmp.tile([128, S], f32, name="vps", tag="mm")
                    for (s0, sn) in SSPL:
                        for p in range(PD):
                            nc.tensor.matmul(vps[:, s0:s0 + sn], wv_s[:, p, hp, :, :],
                                             z1[:, 2 * p:2 * p + 2, s0:s0 + sn],
                                             start=(p == 0), stop=(p == PD - 1),
                                             perf_mode=DR)
                    nc.vector.tensor_scalar(vt[:, hp, :], vps[:, 0:S], DSC_QKV,
                                            bvs[:, hp:hp + 1], op0=ALU.mult, op1=ALU.add)
                # V transposed back to natural [token, feature] layout via PE
                v = qkp.tile([128, len(TCH), D], fp16, name="v", tag="v")
                for hp in range(NHP):
                    for it, (t0, tw) in enumerate(TCH):
                        tps = mmp.tile([128, 128], fp16, name="tps", tag="mm")
                        nc.tensor.transpose(tps[0:tw, 0:128], vt[:, hp, t0:t0 + tw],
                                            idn_s[:, :])
                        nc.vector.tensor_scalar_mul(v[0:tw, it, hp * 128:(hp + 1) * 128],
                                                    tps[0:tw, 0:128], 1.0)

                # --- per-head-pair attention (fp16) ---
                concat = qkp.tile([128, NCD, S1], fp8, name="concat", tag="concat")
                for hp in range(NHP):
                    etiles = [None, None]
                    rstiles = [None, None]
                    for h2 in range(2):
                        hb = h2 * 64
                        e = ep.tile([128, len(TCH), S], fp16, name="e", tag="e")
                        rs = sp_.tile([128, len(TCH)], f32, name="rs", tag="rs", bufs=2)
                        for it, (t0, tw) in enumerate(TCH):
                            stps = mmp.tile([128, S], f32, name="stps", tag="mm")
                            for (s0, sn) in SSPL:
                                nc.tensor.matmul(stps[0:tw, s0:s0 + sn],
                                                 kt[hb:hb + 64, hp, t0:t0 + tw],
                                                 qt[hb:hb + 64, hp, s0:s0 + sn],
                                                 start=True, stop=True)
                            nc.scalar.activation(e[0:tw, it, :], stps[0:tw, 0:S], AF.Exp,
                                                 bias=0.0, scale=float(1.0 / np.sqrt(DH)),
                                                 accum_out=rs[0:tw, it:it + 1])
                        etiles[h2] = e
                        rstiles[h2] = rs
                    ap_ps = mmp.tile([128, S], f32, name="ap_ps", tag="mm",
                                     padded_shape=[128, 1024])
                    for h2 in range(2):
                        hb = h2 * 64
                        e, rs = etiles[h2], rstiles[h2]
                        rec = sp_.tile([128, len(TCH)], f32, name="rec", tag="rec", bufs=2)
                        nfull = len(TCH) - 1
                        nc.vector.reciprocal(rec[:, 0:nfull], rs[:, 0:nfull])
                        lt0, ltw = TCH[-1]
                        nc.vector.reciprocal(rec[0:ltw, nfull:nfull + 1],
                                             rs[0:ltw, nfull:nfull + 1])
                        for it, (t0, tw) in enumerate(TCH):
                            nc.vector.tensor_scalar_mul(v[0:tw, it, hp * 128 + hb:hp * 128 + hb + 64],
                                                        v[0:tw, it, hp * 128 + hb:hp * 128 + hb + 64],
                                                        rec[0:tw, it:it + 1])
                        for (s0, sn) in SSPL:
                            for it, (t0, tw) in enumerate(TCH):
                                nc.tensor.matmul(ap_ps[hb:hb + 64, s0:s0 + sn],
                                                 v[0:tw, it, hp * 128 + hb:hp * 128 + hb + 64],
                                                 e[0:tw, it, s0:s0 + sn],
                                                 start=(it == 0), stop=(it == len(TCH) - 1))
                    nc.vector.tensor_scalar_mul(concat[:, hp, 0:S], ap_ps[:, 0:S], SZ)
                    if interleave:
                        interleave.pop(0)()

                # --- output projection (fp8 DR) + fp16 residual (x+bo re-read
                # from DRAM); LN2 stats matmuls fused into the same chunk loop ---
                dum = sp_.tile([1, 1], f32, name="dum", tag="dum")
                nc.scalar.activation(dum[:, :], eps_s[0:1, 0:1], AF.Sqrt)
                x2 = rp.tile([128, NCD, S], fp16, name="x2", tag="res")
                spt = mmp.tile([128, S], f32, name="spt2", tag="mm", padded_shape=[128, 1024])
                for ec in range(NCD):
                    xres = tp.tile([128, S], fp16, name="xres", tag="castsq", bufs=2)
                    nc.sync.dma_start(out=xres[:, :],
                                      in_=xTb_d[b, ec * 128:(ec + 1) * 128, :])
                    wops = mmp.tile([128, S], f32, name="wops", tag="mm")
                    for (s0, sn) in SSPL:
                        for p in range(PD):
                            nc.tensor.matmul(wops[:, s0:s0 + sn],
                                             wo_s[:, p, ec, :, :],
                                             concat[:, 2 * p:2 * p + 2, s0:s0 + sn],
                                             start=(p == 0), stop=(p == PD - 1),
                                             perf_mode=DR)
                    nc.vector.scalar_tensor_tensor(x2[:, ec, :], wops[:, 0:S],
                                                   DSC_QKV, xres[:, :],
                                                   op0=ALU.mult, op1=ALU.add)
                    sq = emit_squares(x2, ec)
                    for (s0, sn) in SSPL:
                        nc.tensor.matmul(spt[0:1, s0:s0 + sn], ones128[:, :],
                                         x2[:, ec, s0:s0 + sn],
                                         start=(ec == 0), stop=(ec == NCD - 1))
                        nc.tensor.matmul(spt[32:33, s0:s0 + sn], ones128[:, :],
                                         sq[:, s0:s0 + sn],
                                         start=(ec == 0), stop=(ec == NCD - 1))
                x2_tiles[b] = x2
                return spt

            def emit_stats2(b, spt):
                ch2[b] = emit_chain(spt, "2")

            def emit_zfinish2(b):
                rstd_bf, nmr_bf = ch2[b]
                z2row = sixteen1 if fc1_fp8 else ones1
                rbc = emit_bcast(rstd_bf, z2row)
                nbc = emit_bcast(nmr_bf, z2row)
                z2 = zp.tile([128, NCD, S1], fp8 if fc1_fp8 else fp16, name="z2", tag="z")
                emit_znorm(x2_tiles[b], rbc, nbc, z2)
                z2_tiles[b] = z2

            def emit_fc1(b, g, lo, hi):
                z2 = z2_tiles[b]
                for fc in range(lo, hi):
                    fps = mmp.tile([128, S], f32, name="fps", tag="mm")
                    if fc1_fp8:
                        for (s0, sn) in SSPL:
                            for p in range(PD):
                                nc.tensor.matmul(fps[:, s0:s0 + sn],
                                                 w1_s[:, p, fc, :, :],
                                                 z2[:, 2 * p:2 * p + 2, s0:s0 + sn],
                                                 start=(p == 0), stop=(p == PD - 1),
                                                 perf_mode=DR)
                    else:
                        for (s0, sn) in SSPL:
                            for c in range(NCD):
                                nc.tensor.matmul(fps[:, s0:s0 + sn],
                                                 w1_s[:, c, fc * 128:(fc + 1) * 128],
                                                 z2[:, c, s0:s0 + sn],
                                                 start=(c == 0), stop=(c == NCD - 1))
                    nc.scalar.activation(g[:, fc, 0:S], fps[:, 0:S], GELU,
                                         bias=b1s[:, fc:fc + 1], scale=float(dsc_fc1))

            def emit_fc2_chunk(b, g, ec):
                x2 = x2_tiles[b]
                p2 = mmp.tile([128, S], f32, name="p2", tag="mm")
                for (s0, sn) in SSPL:
                    for p in range(PF):
                        nc.tensor.matmul(p2[:, s0:s0 + sn],
                                         w2_s[:, p, ec, :, :],
                                         g[:, 2 * p:2 * p + 2, s0:s0 + sn],
                                         start=(p == 0), stop=(p == PF - 1),
                                         perf_mode=DR)
                nc.vector.scalar_tensor_tensor(x2[:, ec, :], p2[:, 0:S],
                                               DSC_FC2, x2[:, ec, :],
                                               op0=ALU.mult, op1=ALU.add)
                nc.sync.dma_start(out=outT_d[b, ec * 128:(ec + 1) * 128, :],
                                  in_=x2[:, ec, :])

            # ---------------- emission schedule ----------------
            # Two-deep pipeline.  The LN2 chain (serial [1,S] scalar ops) is
            # the exposed latency: everything after attention(b+1) depends on
            # it, so the post-stats2 window is packed with independent PE work
            # (stats1(b+2) + all six FC2(b) chunks) and the sqrt ACT table is
            # preloaded during Wo.  chain1(b+1) hides under FC1(b).
            emit_load_x(0)
            emit_load_weights()
            emit_stats1(0)          # chain1(0) — exposed at startup only
            emit_zfinish1(0)
            spt2 = emit_attention(0)
            emit_stats2(0, spt2)    # chain2(0) — covered by stats1(1)
            emit_load_x(1)
            emit_stats1(1)
            for b in range(bpc):
                emit_zfinish2(b)
                g = gp.tile([128, NCF, S1], fp8, name="g", tag="g")
                emit_fc1(b, g, 0, NCF // 2)
                if b + 1 < bpc:
                    emit_fc1(b, g, NCF // 2, 3 * NCF // 4)
                    emit_zfinish1(b + 1)    # z1(b+1) DVE hides under FC1 tail
                    emit_fc1(b, g, 3 * NCF // 4, NCF)
                    spt2 = emit_attention(b + 1)
                    emit_stats2(b + 1, spt2)
                    if b + 2 < bpc:
                        emit_load_x(b + 2)
                        emit_stats1(b + 2)  # PE cover for chain2(b+1)
                    for ec in range(NCD):
                        emit_fc2_chunk(b, g, ec)
                else:
                    emit_fc1(b, g, NCF // 2, NCF)
                    for ec in range(NCD):
                        emit_fc2_chunk(b, g, ec)
    nc.finalize()
    return nc


def _get_nc(gelu_kind: str = "gelu", bpc: int = BPC, fc1_fp8: bool = False):
    key = (gelu_kind, bpc, fc1_fp8)
    if key not in _NC_CACHE:
        _NC_CACHE[key] = _build_nc(gelu_kind, bpc, fc1_fp8)
    return _NC_CACHE[key]


def _dr_pack(Wsc):
    """Pack a scaled [Din, Dout] f32 weight into the contiguous-pair layout
    dual-fp8 ldweights requires: [128, Din/256, Dout/128, 2, 128] fp8."""
    fp8 = ml_dtypes.float8_e4m3
    Din, Dout = Wsc.shape
    a = Wsc.reshape(Din // 256, 2, 128, Dout // 128, 128)   # [p, j, r, m, c]
    a = a.transpose(2, 0, 3, 1, 4)                          # [r, p, m, j, c]
    return np.ascontiguousarray(a.astype(fp8))


def _prep_weights(inputs, fc1_fp8: bool = False):
    fp16 = np.float16
    f32 = np.float32
    Wq, Wk, Wv = inputs["Wq"], inputs["Wk"], inputs["Wv"]
    g1, b1_ln = np.asarray(inputs["ln1_g"], f32), np.asarray(inputs["ln1_b"], f32)
    g2, b2_ln = np.asarray(inputs["ln2_g"], f32), np.asarray(inputs["ln2_b"], f32)

    def flat(Wx):  # [H, D, DH] -> [D, H*DH]
        return np.ascontiguousarray(np.transpose(np.asarray(Wx, f32), (1, 0, 2)).reshape(D, D))

    wq_f, wk_f, wv_f = flat(Wq), flat(Wk), flat(Wv)
    W1 = np.asarray(inputs["W1"], f32)
    w1_scaled = g2[:, None] * W1
    out = {
        "wq": _dr_pack(g1[:, None] * wq_f * SW),
        "wk": _dr_pack(g1[:, None] * wk_f * SW),
        "wv": _dr_pack(g1[:, None] * wv_f * SW),
        "wo": _dr_pack(np.asarray(inputs["Wo"], f32) * SW),
        "w1": _dr_pack(w1_scaled * SW) if fc1_fp8
              else np.ascontiguousarray(w1_scaled.astype(fp16)),
        "w2": _dr_pack(np.asarray(inputs["W2"], f32) * SW),
        "bq": (b1_ln @ wq_f + np.asarray(inputs["bq"], f32).reshape(-1)).reshape(NCD, 128).astype(f32),
        "bk": (b1_ln @ wk_f + np.asarray(inputs["bk"], f32).reshape(-1)).reshape(NCD, 128).astype(f32),
        "bv": (b1_ln @ wv_f + np.asarray(inputs["bv"], f32).reshape(-1)).reshape(NCD, 128).astype(f32),
        "b1": (b2_ln @ W1 + np.asarray(inputs["b1"], f32)).reshape(NCF, 128).astype(f32),
        "idn": np.eye(128, dtype=fp16),
    }
    return out


def kernel(**inputs) -> np.ndarray:
    from concourse.bass_utils import run_bass_kernel_spmd

    fc1_fp8 = True
    nc = _get_nc("gelu", BPC, fc1_fp8)
    w = _prep_weights(inputs, fc1_fp8)
    x = np.asarray(inputs["x"], np.float32)
    bo = np.asarray(inputs["bo"], np.float32)
    b2 = np.asarray(inputs["b2"], np.float32)
    # shard over batch, transpose to [b, D, S] per core, fp16 residual stream
    xT = x.reshape(NCORES, BPC, S, D).swapaxes(2, 3)          # [8, BPC, D, S]
    xT16 = np.ascontiguousarray(xT.astype(np.float16))
    xTb16 = np.ascontiguousarray((xT + bo[None, None, :, None]).astype(np.float16))
    in_maps = [dict(w, xT=xT16[i], xTb=xTb16[i]) for i in range(NCORES)]
    res = run_bass_kernel_spmd(nc, in_maps, core_ids=list(range(NCORES)))
    outs = [res.results[i]["outT"] for i in range(NCORES)]   # each [BPC, D, S] fp16
    out = np.stack(outs, 0).astype(np.float32).swapaxes(2, 3).reshape(B, S, D)
    out += b2[None, None, :]
    return np.ascontiguousarray(out)


# revision 15
# speedup vs baseline: 1.1431x; 1.1431x over previous
"""Trainium2 Bass kernel for a dense transformer block.

Reference math (B=32, S=577, D=768, H=12, DH=64, F=3072, fp32):
  h  = LN1(x);  q,k,v = per-head projections of h
  scores = q @ k^T / sqrt(DH)
  probs  = softmax(scores, axis=QUERY)       # quirk: softmax over the query axis
  attn   = probs @ v;  x2 = x + concat(attn) @ Wo + bo
  out    = x2 + (gelu(LN2(x2) @ W1 + b1) @ W2 + b2)

Strategy: pure data-parallel over batch, 4 batch items per core on 8 cores, no
collectives.  On-chip activations live transposed [feature on partitions, token
on free dim].  v2: fp16 residual stream end-to-end (x shipped fp16, halved DMA
and 2x DVE rate), fp8e4m3 DoubleRow matmuls (K=256/instr, 2x PE throughput) for
the QKV projections, output projection and FC2 (FC1 optionally), with
power-of-2 scales (weights x128, normalized activations x16) descaled in the
existing PSUM->SBUF copy ops.  Dual-fp8 ldweights requires the two k-subtiles
contiguous in SBUF, so fp8 weights ship pre-packed as [128, pair, block, 2,
128] and V is projected transposed (like Q/K) then PE-transposed back to the
natural [token, feature] layout the attention kernel needs.  LN stats run as
fp16 ones-matmuls.  bo is folded into a second host-side residual copy of x;
b2 is applied on the host after the gather.
"""

import numpy as np
import ml_dtypes

B, S, D, H, DH, F = 32, 577, 768, 12, 64, 3072
NCORES = 8
BPC = B // NCORES          # batches per core
EPS = 1e-5
NCD = D // 128             # 6  d-chunks
NCF = F // 128             # 24 f-chunks
NHP = H // 2               # 6  head pairs
PD = D // 256              # 3  d chunk-pairs
PF = F // 256              # 12 f chunk-pairs
SSPL = [(0, 512), (512, S - 512)]              # free-dim splits of S for matmul/psum
TCH = [(i * 128, min(128, S - i * 128)) for i in range((S + 127) // 128)]  # 5 t-chunks
S1 = S + (S % 2)           # even-padded free dim for fp8 DoubleRow operand tiles

SW = 128.0                 # fp8 weight scale (power of 2)
SZ = 16.0                  # fp8 normalized-activation scale
DSC_QKV = 1.0 / (SW * SZ)  # psum descale for z1(fp8) @ w(fp8)
DSC_FC2 = 1.0 / SW         # psum descale for g(fp8, unscaled) @ w2(fp8)

_NC_CACHE = {}


def _build_nc(gelu_kind: str = "gelu", bpc: int = BPC, fc1_fp8: bool = False):
    from contextlib import ExitStack
    import concourse.bass as bass
    import concourse.tile as tile
    from concourse import bacc, mybir

    f32, fp16 = mybir.dt.float32, mybir.dt.float16
    fp8 = mybir.dt.float8e4
    AF = mybir.ActivationFunctionType
    ALU = mybir.AluOpType
    DR = mybir.MatmulPerfMode.DoubleRow
    GELU = {"gelu": AF.Gelu, "tanh": AF.Tanh}[gelu_kind]
    z2scale = SZ if fc1_fp8 else 1.0
    dsc_fc1 = DSC_QKV if fc1_fp8 else 1.0

    nc = bacc.Bacc("TRN2", target_bir_lowering=False, dynamic_dma_scratch_size=2048)
    xT_d = nc.declare_dram_parameter("xT", [bpc, D, S], fp16, isOutput=False)
    xTb_d = nc.declare_dram_parameter("xTb", [bpc, D, S], fp16, isOutput=False)
    wq_d = nc.declare_dram_parameter("wq", [128, PD, NCD, 2, 128], fp8, isOutput=False)
    wk_d = nc.declare_dram_parameter("wk", [128, PD, NCD, 2, 128], fp8, isOutput=False)
    wv_d = nc.declare_dram_parameter("wv", [128, PD, NCD, 2, 128], fp8, isOutput=False)
    wo_d = nc.declare_dram_parameter("wo", [128, PD, NCD, 2, 128], fp8, isOutput=False)
    if fc1_fp8:
        w1_d = nc.declare_dram_parameter("w1", [128, PD, NCF, 2, 128], fp8, isOutput=False)
    else:
        w1_d = nc.declare_dram_parameter("w1", [D, F], fp16, isOutput=False)
    w2_d = nc.declare_dram_parameter("w2", [128, PF, NCD, 2, 128], fp8, isOutput=False)
    bq_d = nc.declare_dram_parameter("bq", [NCD, 128], f32, isOutput=False)
    bk_d = nc.declare_dram_parameter("bk", [NCD, 128], f32, isOutput=False)
    bv_d = nc.declare_dram_parameter("bv", [NCD, 128], f32, isOutput=False)
    b1_d = nc.declare_dram_parameter("b1", [NCF, 128], f32, isOutput=False)
    idn_d = nc.declare_dram_parameter("idn", [128, 128], fp16, isOutput=False)
    outT_d = nc.declare_dram_parameter("outT", [bpc, D, S], fp16, isOutput=True)

    with tile.TileContext(nc) as tc:
        with ExitStack() as ctx:
            wp = ctx.enter_context(tc.tile_pool(name="wp", bufs=1))
            rp = ctx.enter_context(tc.tile_pool(name="rp", bufs=2))      # residual fp16
            zp = ctx.enter_context(tc.tile_pool(name="zp", bufs=1))      # normalized
            qkp = ctx.enter_context(tc.tile_pool(name="qkp", bufs=1))    # qt/kt/vt/v/concat
            ep = ctx.enter_context(tc.tile_pool(name="ep", bufs=2))      # exp tiles
            gp = ctx.enter_context(tc.tile_pool(name="gp", bufs=1))      # gelu acts
            sp_ = ctx.enter_context(tc.tile_pool(name="sp", bufs=1))     # small stat rows
            tp = ctx.enter_context(tc.tile_pool(name="tp", bufs=1))      # [128,S] temps
            mmp = ctx.enter_context(tc.tile_pool(name="mmp", bufs=4, space="PSUM"))

            # ---- weights / constants (resident); DMAs deferred until after
            # the first x-shard load so compute starts immediately ----
            wq_s = wp.tile([128, PD, NCD, 2, 128], fp8, name="wq_s")
            wk_s = wp.tile([128, PD, NCD, 2, 128], fp8, name="wk_s")
            wv_s = wp.tile([128, PD, NCD, 2, 128], fp8, name="wv_s")
            wo_s = wp.tile([128, PD, NCD, 2, 128], fp8, name="wo_s")
            if fc1_fp8:
                w1_s = wp.tile([128, PD, NCF, 2, 128], fp8, name="w1_s")
            else:
                w1_s = wp.tile([128, NCD, F], fp16, name="w1_s")
            w2_s = wp.tile([128, PF, NCD, 2, 128], fp8, name="w2_s")

            def emit_load_weights():
                nc.sync.dma_start(out=wq_s[:, :, :, :, :], in_=wq_d[:, :, :, :, :])
                nc.sync.dma_start(out=wk_s[:, :, :, :, :], in_=wk_d[:, :, :, :, :])
                nc.sync.dma_start(out=wv_s[:, :, :, :, :], in_=wv_d[:, :, :, :, :])
                nc.sync.dma_start(out=wo_s[:, :, :, :, :], in_=wo_d[:, :, :, :, :])
                if fc1_fp8:
                    nc.sync.dma_start(out=w1_s[:, :, :, :, :], in_=w1_d[:, :, :, :, :])
                else:
                    nc.sync.dma_start(out=w1_s[:, :, :], in_=w1_d.ap().rearrange("(c p) n -> p c n", p=128))
                nc.sync.dma_start(out=w2_s[:, :, :, :, :], in_=w2_d[:, :, :, :, :])
            bqs = wp.tile([128, NCD], f32, name="bqs")
            nc.sync.dma_start(out=bqs[:, :], in_=bq_d.ap().rearrange("c p -> p c"))
            bks = wp.tile([128, NCD], f32, name="bks")
            nc.sync.dma_start(out=bks[:, :], in_=bk_d.ap().rearrange("c p -> p c"))
            bvs = wp.tile([128, NCD], f32, name="bvs")
            nc.sync.dma_start(out=bvs[:, :], in_=bv_d.ap().rearrange("c p -> p c"))
            b1s = wp.tile([128, NCF], f32, name="b1s")
            nc.sync.dma_start(out=b1s[:, :], in_=b1_d.ap().rearrange("c p -> p c"))
            idn_s = wp.tile([128, 128], fp16, name="idn_s")
            nc.sync.dma_start(out=idn_s[:, :], in_=idn_d[:, :])
            ones128 = wp.tile([128, 1], fp16, name="ones128")
            nc.vector.memset(ones128[:, :], 1.0)
            ones1 = wp.tile([1, 128], fp16, name="ones1")
            nc.vector.memset(ones1[:, :], 1.0)
            sixteen1 = wp.tile([1, 128], fp16, name="sixteen1")
            nc.vector.memset(sixteen1[:, :], SZ)
            eps_s = wp.tile([1, 1], f32, name="eps_s")
            nc.vector.memset(eps_s[:, :], EPS)

            # ---------------- helpers ----------------
            def emit_squares(src, c):
                """ACT square of one chunk of src (fp16) -> fp16 tile for sumsq."""
                sq = tp.tile([128, S], fp16, name="sq", tag="castsq", bufs=2)
                nc.scalar.activation(sq[:, :], src[:, c, :], AF.Square)
                return sq

            def emit_stats(src):
                """Column sums & sums of squares of src [128, NCD, S] fp16 over
                the partition (feature) axis -> psum rows [0]=sum, [32]=sumsq."""
                spt = mmp.tile([128, S], f32, name="spt", tag="mm", padded_shape=[128, 1024])
                sqs = [emit_squares(src, c) for c in range(NCD)]
                for c in range(NCD):
                    for (s0, sn) in SSPL:
                        nc.tensor.matmul(spt[0:1, s0:s0 + sn], ones128[:, :],
                                         src[:, c, s0:s0 + sn],
                                         start=(c == 0), stop=(c == NCD - 1))
                        nc.tensor.matmul(spt[32:33, s0:s0 + sn], ones128[:, :],
                                         sqs[c][:, s0:s0 + sn],
                                         start=(c == 0), stop=(c == NCD - 1))
                return spt

            def emit_chain(spt):
                """LN scalar chain on [1,S] rows, minimized for serial depth:
                mu2 -> var -> rstd(ACT Rsqrt) -> nmr, with mu in parallel.
                The fp8 activation scale is folded into the bcast lhsT."""
                mu_s = sp_.tile([1, S], f32, name="mu_s", tag="mu_s")
                nc.vector.tensor_scalar_mul(mu_s[:, :], spt[0:1, :], 1.0 / D)
                mu2 = sp_.tile([1, S], f32, name="mu2", tag="v_s")
                nc.vector.tensor_mul(mu2[:, :], mu_s[:, :], mu_s[:, :])
                var = tp.tile([1, S], f32, name="var", tag="castsq", bufs=2)
                nc.vector.scalar_tensor_tensor(var[:, :], spt[32:33, :], 1.0 / D,
                                               mu2[:, :], op0=ALU.mult, op1=ALU.subtract)
                w_s = tp.tile([1, S], f32, name="w_s", tag="castsq", bufs=2)
                nc.scalar.activation(w_s[:, :], var[:, :], AF.Sqrt, bias=eps_s[0:1, 0:1])
                v_s = sp_.tile([1, S], f32, name="v_s2", tag="v_s2")
                nc.vector.reciprocal_approx_fast(v_s[:, :], w_s[:, :])
                rstd_bf = sp_.tile([1, S], fp16, name="rstd_bf", tag="rstdbf")
                nc.scalar.activation(rstd_bf[:, :], v_s[:, :], AF.Copy)
                nmr_bf = sp_.tile([1, S], fp16, name="nmr_bf", tag="nmrbf")
                nc.vector.scalar_tensor_tensor(nmr_bf[:, :], mu_s[:, :], -1.0, v_s[:, :],
                                               op0=ALU.mult, op1=ALU.mult)
                return rstd_bf, nmr_bf

            def emit_bcast(row_bf, lhs_row):
                """Broadcast a [1,S] fp16 row across 128 partitions via rank-1
                matmul; lhs_row's value carries the fp8 activation scale."""
                bc = mmp.tile([128, D], f32, name="bc", tag="mm")
                for (s0, sn) in SSPL:
                    nc.tensor.matmul(bc[:, s0:s0 + sn], lhs_row[0:1, :], row_bf[0:1, s0:s0 + sn],
                                     start=True, stop=True)
                return bc

            def emit_znorm(src, rstd_bc, nmr_bc, z):
                """z[:,c,:] = (src[:,c,:] * rstd) + negmurstd."""
                for c in range(NCD):
                    zt = tp.tile([128, S], fp16, name="zt", tag="castsq", bufs=2)
                    nc.vector.tensor_mul(zt[:, :], src[:, c, :], rstd_bc[:, 0:S])
                    nc.vector.tensor_add(z[:, c, 0:S], zt[:, :], nmr_bc[:, 0:S])

            # ---------------- phase emitters ----------------
            xt_tiles = [None] * bpc
            z1_tiles = [None] * bpc
            z2_tiles = [None] * bpc
            x2_tiles = [None] * bpc
            ch1 = [None] * bpc
            ch2 = [None] * bpc

            def emit_load_x(b):
                xt = rp.tile([128, NCD, S], fp16, name="xt", tag="res")
                for c in range(NCD):
                    nc.sync.dma_start(out=xt[:, c, :],
                                      in_=xT_d[b, c * 128:(c + 1) * 128, :])
                xt_tiles[b] = xt

            def emit_stats1(b):
                ch1[b] = emit_chain(emit_stats(xt_tiles[b]))

            def emit_zfinish1(b):
                rstd_bf, nmr_bf = ch1[b]
                rbc = emit_bcast(rstd_bf, sixteen1)
                nbc = emit_bcast(nmr_bf, sixteen1)
                z1 = zp.tile([128, NCD, S1], fp8, name="z1", tag="z")
                emit_znorm(xt_tiles[b], rbc, nbc, z1)
                z1_tiles[b] = z1

            def emit_attention(b, interleave=()):
                """interleave: callables emitted between head-pairs to feed the
                PE while ACT grinds the softmax exps."""
                interleave = list(interleave)
                z1 = z1_tiles[b]
                # --- QKV projections (fp8 DoubleRow over chunk pairs) ---
                qt = qkp.tile([128, NHP, S], fp16, name="qt", tag="qt")
                kt = qkp.tile([128, NHP, S], fp16, name="kt", tag="kt")
                vt = qkp.tile([128, NHP, S], fp16, name="vt", tag="vt")
                for hp in range(NHP):
                    qps = mmp.tile([128, S], f32, name="qps", tag="mm")
                    for (s0, sn) in SSPL:
                        for p in range(PD):
                            nc.tensor.matmul(qps[:, s0:s0 + sn], wq_s[:, p, hp, :, :],
                                             z1[:, 2 * p:2 * p + 2, s0:s0 + sn],
                                             start=(p == 0), stop=(p == PD - 1),
                                             perf_mode=DR)
                    nc.vector.tensor_scalar(qt[:, hp, :], qps[:, 0:S], DSC_QKV,
                                            bqs[:, hp:hp + 1], op0=ALU.mult, op1=ALU.add)
                    kps = mmp.tile([128, S], f32, name="kps", tag="mm")
                    for (s0, sn) in SSPL:
                        for p in range(PD):
                            nc.tensor.matmul(kps[:, s0:s0 + sn], wk_s[:, p, hp, :, :],
                                             z1[:, 2 * p:2 * p + 2, s0:s0 + sn],
                                             start=(p == 0), stop=(p == PD - 1),
                                             perf_mode=DR)
                    nc.vector.tensor_scalar(kt[:, hp, :], kps[:, 0:S], DSC_QKV,
                                            bks[:, hp:hp + 1], op0=ALU.mult, op1=ALU.add)
                    vps = mmp.tile([128, S], f32, name="vps", tag="mm")
                    for (s0, sn) in SSPL:
                        for p in range(PD):
                            nc.tensor.matmul(vps[:, s0:s0 + sn], wv_s[:, p, hp, :, :],
                                             z1[:, 2 * p:2 * p + 2, s0:s0 + sn],
                                             start=(p == 0), stop=(p == PD - 1),
                                             perf_mode=DR)
                    nc.vector.tensor_scalar(vt[:, hp, :], vps[:, 0:S], DSC_QKV,
                                            bvs[:, hp:hp + 1], op0=ALU.mult, op1=ALU.add)
                # V transposed back to natural [token, feature] layout via PE
                v = qkp.tile([128, len(TCH), D], fp16, name="v", tag="v")
                for hp in range(NHP):
                    for it, (t0, tw) in enumerate(TCH):
                        tps = mmp.tile([128, 128], fp16, name="tps", tag="mm")
                        nc.tensor.transpose(tps[0:tw, 0:128], vt[:, hp, t0:t0 + tw],
                                            idn_s[:, :])
                        nc.vector.tensor_scalar_mul(v[0:tw, it, hp * 128:(hp + 1) * 128],
                                                    tps[0:tw, 0:128], 1.0)

                # --- per-head-pair attention (fp16) ---
                concat = qkp.tile([128, NCD, S1], fp8, name="concat", tag="concat")
                for hp in range(NHP):
                    etiles = [None, None]
                    rstiles = [None, None]
                    for h2 in range(2):
                        hb = h2 * 64
                        e = ep.tile([128, len(TCH), S], fp16, name="e", tag="e")
                        rs = sp_.tile([128, len(TCH)], f32, name="rs", tag="rs", bufs=2)
                        for it, (t0, tw) in enumerate(TCH):
                            stps = mmp.tile([128, S], f32, name="stps", tag="mm")
                            for (s0, sn) in SSPL:
                                nc.tensor.matmul(stps[0:tw, s0:s0 + sn],
                                                 kt[hb:hb + 64, hp, t0:t0 + tw],
                                                 qt[hb:hb + 64, hp, s0:s0 + sn],
                                                 start=True, stop=True)
                            nc.scalar.activation(e[0:tw, it, :], stps[0:tw, 0:S], AF.Exp,
                                                 bias=0.0, scale=float(1.0 / np.sqrt(DH)),
                                                 accum_out=rs[0:tw, it:it + 1])
                        etiles[h2] = e
                        rstiles[h2] = rs
                    ap_ps = mmp.tile([128, S], f32, name="ap_ps", tag="mm",
                                     padded_shape=[128, 1024])
                    for h2 in range(2):
                        hb = h2 * 64
                        e, rs = etiles[h2], rstiles[h2]
                        rec = sp_.tile([128, len(TCH)], f32, name="rec", tag="rec", bufs=2)
                        nfull = len(TCH) - 1
                        nc.vector.reciprocal(rec[:, 0:nfull], rs[:, 0:nfull])
                        lt0, ltw = TCH[-1]
                        nc.vector.reciprocal(rec[0:ltw, nfull:nfull + 1],
                                             rs[0:ltw, nfull:nfull + 1])
                        for it, (t0, tw) in enumerate(TCH):
                            nc.vector.tensor_scalar_mul(v[0:tw, it, hp * 128 + hb:hp * 128 + hb + 64],
                                                        v[0:tw, it, hp * 128 + hb:hp * 128 + hb + 64],
                                                        rec[0:tw, it:it + 1])
                        for (s0, sn) in SSPL:
                            for it, (t0, tw) in enumerate(TCH):
                                nc.tensor.matmul(ap_ps[hb:hb + 64, s0:s0 + sn],
                                                 v[0:tw, it, hp * 128 + hb:hp * 128 + hb + 64],
                                                 e[0:tw, it, s0:s0 + sn],
                                                 start=(it == 0), stop=(it == len(TCH) - 1))
                    nc.vector.tensor_scalar_mul(concat[:, hp, 0:S], ap_ps[:, 0:S], SZ)
                    if interleave:
                        interleave.pop(0)()

                # --- output projection (fp8 DR) + fp16 residual (x+bo re-read
                # from DRAM); LN2 stats matmuls fused into the same chunk loop ---
                x2 = rp.tile([128, NCD, S], fp16, name="x2", tag="res")
                spt = mmp.tile([128, S], f32, name="spt2", tag="mm", padded_shape=[128, 1024])
                for ec in range(NCD):
                    xres = tp.tile([128, S], fp16, name="xres", tag="castsq", bufs=2)
                    nc.sync.dma_start(out=xres[:, :],
                                      in_=xTb_d[b, ec * 128:(ec + 1) * 128, :])
                    wops = mmp.tile([128, S], f32, name="wops", tag="mm")
                    for (s0, sn) in SSPL:
                        for p in range(PD):
                            nc.tensor.matmul(wops[:, s0:s0 + sn],
                                             wo_s[:, p, ec, :, :],
                                             concat[:, 2 * p:2 * p + 2, s0:s0 + sn],
                                             start=(p == 0), stop=(p == PD - 1),
                                             perf_mode=DR)
                    nc.vector.scalar_tensor_tensor(x2[:, ec, :], wops[:, 0:S],
                                                   DSC_QKV, xres[:, :],
                                                   op0=ALU.mult, op1=ALU.add)
                    sq = emit_squares(x2, ec)
                    for (s0, sn) in SSPL:
                        nc.tensor.matmul(spt[0:1, s0:s0 + sn], ones128[:, :],
                                         x2[:, ec, s0:s0 + sn],
                                         start=(ec == 0), stop=(ec == NCD - 1))
                        nc.tensor.matmul(spt[32:33, s0:s0 + sn], ones128[:, :],
                                         sq[:, s0:s0 + sn],
                                         start=(ec == 0), stop=(ec == NCD - 1))
                x2_tiles[b] = x2
                return spt

            def emit_stats2(b, spt):
                ch2[b] = emit_chain(spt)

            def emit_zfinish2(b):
                rstd_bf, nmr_bf = ch2[b]
                z2row = sixteen1 if fc1_fp8 else ones1
                rbc = emit_bcast(rstd_bf, z2row)
                nbc = emit_bcast(nmr_bf, z2row)
                z2 = zp.tile([128, NCD, S1], fp8 if fc1_fp8 else fp16, name="z2", tag="z")
                emit_znorm(x2_tiles[b], rbc, nbc, z2)
                z2_tiles[b] = z2

            def emit_fc1(b, g, lo, hi):
                z2 = z2_tiles[b]
                for fc in range(lo, hi):
                    fps = mmp.tile([128, S], f32, name="fps", tag="mm")
                    if fc1_fp8:
                        for (s0, sn) in SSPL:
                            for p in range(PD):
                                nc.tensor.matmul(fps[:, s0:s0 + sn],
                                                 w1_s[:, p, fc, :, :],
                                                 z2[:, 2 * p:2 * p + 2, s0:s0 + sn],
                                                 start=(p == 0), stop=(p == PD - 1),
                                                 perf_mode=DR)
                    else:
                        for (s0, sn) in SSPL:
                            for c in range(NCD):
                                nc.tensor.matmul(fps[:, s0:s0 + sn],
                                                 w1_s[:, c, fc * 128:(fc + 1) * 128],
                                                 z2[:, c, s0:s0 + sn],
                                                 start=(c == 0), stop=(c == NCD - 1))
                    nc.scalar.activation(g[:, fc, 0:S], fps[:, 0:S], GELU,
                                         bias=b1s[:, fc:fc + 1], scale=float(dsc_fc1))

            def emit_fc2_chunk(b, g, ec):
                x2 = x2_tiles[b]
                p2 = mmp.tile([128, S], f32, name="p2", tag="mm")
                for (s0, sn) in SSPL:
                    for p in range(PF):
                        nc.tensor.matmul(p2[:, s0:s0 + sn],
                                         w2_s[:, p, ec, :, :],
                                         g[:, 2 * p:2 * p + 2, s0:s0 + sn],
                                         start=(p == 0), stop=(p == PF - 1),
                                         perf_mode=DR)
                nc.vector.scalar_tensor_tensor(x2[:, ec, :], p2[:, 0:S],
                                               DSC_FC2, x2[:, ec, :],
                                               op0=ALU.mult, op1=ALU.add)
                nc.sync.dma_start(out=outT_d[b, ec * 128:(ec + 1) * 128, :],
                                  in_=x2[:, ec, :])

            # ---------------- emission schedule ----------------
            # Two-deep pipeline: FC2(b) chunks 0-2 are interleaved between
            # attention(b+1) head-pairs (ACT-bound there) and chunks 3-5 land
            # after the fused Wo/LN2-stats so batch b+1's LN2 chain hides
            # under their matmuls; chain1(b+1) hides under FC1(b).
            emit_load_x(0)
            emit_load_weights()
            emit_stats1(0)          # chain1(0) — exposed at startup only
            emit_zfinish1(0)
            spt2 = emit_attention(0)
            emit_stats2(0, spt2)    # chain2(0) — exposed once (no prior FC2)
            for b in range(bpc):
                if b + 1 < bpc:
                    emit_load_x(b + 1)
                emit_zfinish2(b)
                g = gp.tile([128, NCF, S1], fp8, name="g", tag="g")
                emit_fc1(b, g, 0, NCF // 2)
                if b + 1 < bpc:
                    emit_stats1(b + 1)      # chain1(b+1) runs during FC1(b)
                    emit_fc1(b, g, NCF // 2, 3 * NCF // 4)
                    emit_zfinish1(b + 1)    # z1(b+1) DVE hides under FC1 tail
                    emit_fc1(b, g, 3 * NCF // 4, NCF)
                    il = [(lambda ec=ec: emit_fc2_chunk(b, g, ec)) for ec in range(3)]
                    spt2 = emit_attention(b + 1, interleave=il)
                    emit_stats2(b + 1, spt2)
                    emit_fc2_chunk(b, g, 3)
                    emit_fc2_chunk(b, g, 4)
                    emit_fc2_chunk(b, g, 5)
                else:
                    emit_fc1(b, g, NCF // 2, NCF)
                    for ec in range(NCD):
                        emit_fc2_chunk(b, g, ec)
    nc.finalize()
    return nc


def _get_nc(gelu_kind: str = "gelu", bpc: int = BPC, fc1_fp8: bool = False):
    key = (gelu_kind, bpc, fc1_fp8)
    if key not in _NC_CACHE:
        _NC_CACHE[key] = _build_nc(gelu_kind, bpc, fc1_fp8)
    return _NC_CACHE[key]


def _dr_pack(Wsc):
    """Pack a scaled [Din, Dout] f32 weight into the contiguous-pair layout
    dual-fp8 ldweights requires: [128, Din/256, Dout/128, 2, 128] fp8."""
    fp8 = ml_dtypes.float8_e4m3
    Din, Dout = Wsc.shape
    a = Wsc.reshape(Din // 256, 2, 128, Dout // 128, 128)   # [p, j, r, m, c]
    a = a.transpose(2, 0, 3, 1, 4)                          # [r, p, m, j, c]
    return np.ascontiguousarray(a.astype(fp8))


def _prep_weights(inputs, fc1_fp8: bool = False):
    fp16 = np.float16
    f32 = np.float32
    Wq, Wk, Wv = inputs["Wq"], inputs["Wk"], inputs["Wv"]
    g1, b1_ln = np.asarray(inputs["ln1_g"], f32), np.asarray(inputs["ln1_b"], f32)
    g2, b2_ln = np.asarray(inputs["ln2_g"], f32), np.asarray(inputs["ln2_b"], f32)

    def flat(Wx):  # [H, D, DH] -> [D, H*DH]
        return np.ascontiguousarray(np.transpose(np.asarray(Wx, f32), (1, 0, 2)).reshape(D, D))

    wq_f, wk_f, wv_f = flat(Wq), flat(Wk), flat(Wv)
    W1 = np.asarray(inputs["W1"], f32)
    w1_scaled = g2[:, None] * W1
    out = {
        "wq": _dr_pack(g1[:, None] * wq_f * SW),
        "wk": _dr_pack(g1[:, None] * wk_f * SW),
        "wv": _dr_pack(g1[:, None] * wv_f * SW),
        "wo": _dr_pack(np.asarray(inputs["Wo"], f32) * SW),
        "w1": _dr_pack(w1_scaled * SW) if fc1_fp8
              else np.ascontiguousarray(w1_scaled.astype(fp16)),
        "w2": _dr_pack(np.asarray(inputs["W2"], f32) * SW),
        "bq": (b1_ln @ wq_f + np.asarray(inputs["bq"], f32).reshape(-1)).reshape(NCD, 128).astype(f32),
        "bk": (b1_ln @ wk_f + np.asarray(inputs["bk"], f32).reshape(-1)).reshape(NCD, 128).astype(f32),
        "bv": (b1_ln @ wv_f + np.asarray(inputs["bv"], f32).reshape(-1)).reshape(NCD, 128).astype(f32),
        "b1": (b2_ln @ W1 + np.asarray(inputs["b1"], f32)).reshape(NCF, 128).astype(f32),
        "idn": np.eye(128, dtype=fp16),
    }
    return out


def kernel(**inputs) -> np.ndarray:
    from concourse.bass_utils import run_bass_kernel_spmd

    fc1_fp8 = True
    nc = _get_nc("gelu", BPC, fc1_fp8)
    w = _prep_weights(inputs, fc1_fp8)
    x = np.asarray(inputs["x"], np.float32)
    bo = np.asarray(inputs["bo"], np.float32)
    b2 = np.asarray(inputs["b2"], np.float32)
    # shard over batch, transpose to [b, D, S] per core, fp16 residual stream
    xT = x.reshape(NCORES, BPC, S, D).swapaxes(2, 3)          # [8, BPC, D, S]
    xT16 = np.ascontiguousarray(xT.astype(np.float16))
    xTb16 = np.ascontiguousarray((xT + bo[None, None, :, None]).astype(np.float16))
    in_maps = [dict(w, xT=xT16[i], xTb=xTb16[i]) for i in range(NCORES)]
    res = run_bass_kernel_spmd(nc, in_maps, core_ids=list(range(NCORES)))
    outs = [res.results[i]["outT"] for i in range(NCORES)]   # each [BPC, D, S] fp16
    out = np.stack(outs, 0).astype(np.float32).swapaxes(2, 3).reshape(B, S, D)
    out += b2[None, None, :]
    return np.ascontiguousarray(out)
